# revision 8
# baseline (speedup 1.0000x reference)
"""Trainium2 Bass kernel for nn_HKANGNN (hetero GraphConv + KAN head).

Math (only the email-node output path matters):
  e    = x_email @ w_email.T + b_email
  agg_se[n] = sum_{se edges -> n} (x_sender[src] @ w_sender.T + b_sender)
  agg_ue[n] = sum_{ue edges -> n} (x_url[src]    @ w_url.T    + b_url)
  out_e = agg_se @ w_rel_se.T + b_rel_se + agg_ue @ w_rel_ue.T + b_rel_ue
        + e @ (w_root_se + w_root_ue).T
  h = relu(out_e);  out = silu(h) @ base_w.T + einsum(b_splines(h), spline_w)

Device strategy (8 cores, email nodes sharded 12500/core, padded to 12800):
  * by linearity the per-edge payload is the RAW source features; the tiny
    projections fold into mcAgg on host (biases are zero -> no count terms).
  * edges per class (sender / urlA / urlB) are dst-sorted into a FLAT stream,
    padded to x128 at 512-node page boundaries (~20% pad).
  * dma_gather fetches 256B rows per edge (Q7-descriptor-bound ~8ns/edge);
    scatter is one fp8 one-hot matmul [K<=128 x 512n] per part (page-run of a
    128-edge group) into a per-page PSUM accumulator; the chunk/page/part
    schedule depends only on cross-core maxima -> SPMD-uniform.
  * projection: out_e.T accumulated in PSUM over 6 K-chunks of
    (Wrootsum@w_email).T (bf16) + mcAgg @ page.
  * KAN head: spline(h) == q0 + q1 x + q2 x^2 + q3 x^3 + sum_k W'_k relu(x-t_k)^3
    with x = clamp(h,0,2.2); chunks stay f32 (power-basis coefficients amplify
    2-byte rounding); knot relus on the scalar engine, cubes as 1-port DVE
    tensor_tensor (avoids SWDGE/DVE 2-port contention); q0 added in the
    output copy.
"""

import os
import numpy as np
import ml_dtypes

import concourse.bass as bass
import concourse.mybir as mybir
import concourse.tile as tile
from concourse import bacc
from concourse.bass_utils import run_bass_kernel_spmd

F32 = mybir.dt.float32
F16 = mybir.dt.float16
BF16 = mybir.dt.bfloat16
FP8 = mybir.dt.float8e4
I16 = mybir.dt.int16
BF = ml_dtypes.bfloat16
F8 = ml_dtypes.float8_e4m3

N_CORES = 8
HID = 128
NE, NS, NU = 100000, 30000, 50000
NSH = NE // N_CORES          # 12500 real nodes per core
NP = 12800                   # padded: 25 pages x 512 nodes
PAGES = NP // 512
KIN = 768
NKC = KIN // 128             # 6 projection K-chunks
URL_SPLIT = 25600            # url class A rows [0,25600), B rows [25600,50000)
ELEM = 128                   # gather row: 128 bf16 = 256 B
CH_MAX = 4096                # max edges per dma_gather call
KNOTS = (0.2, 0.6, 1.0, 1.4, 1.8)
XCLAMP = 2.2

_LAST_RESULT = None
_CACHE = {}


# ----------------------------------------------------------------- host folds
def _head_weights(base_w, spline_w):
    """[128, 20] f32: lhsT ([d,2]) per head chunk, order
    [silu, ones, x, x^2, x^3, R(.2)^3, R(.6)^3, R(1.0)^3, R(1.4)^3, R(1.8)^3]."""
    c = np.array([1.0, -4.0, 6.0, -4.0, 1.0], np.float64)
    h = 0.4
    scale = 1.0 / (6.0 * h ** 3)
    O, D, B = spline_w.shape                      # [2, 128, 8]
    wp = np.zeros((O, D, 11), np.float64)         # W'[o,d,m], m=0..10
    for m in range(11):
        for j in range(5):
            b = m - j
            if 0 <= b < B:
                wp[:, :, m] += spline_w[:, :, b].astype(np.float64) * c[j] * scale
    t = np.arange(11) * h - 2.2                   # knot m at t_m
    q = np.zeros((4, O, D), np.float64)           # poly coeffs from m=0..5
    for m in range(6):
        q[0] += -t[m] ** 3 * wp[:, :, m]
        q[1] += 3 * t[m] ** 2 * wp[:, :, m]
        q[2] += -3 * t[m] * wp[:, :, m]
        q[3] += wp[:, :, m]
    head = np.zeros((D, 20), np.float64)
    head[:, 0:2] = base_w.T                       # silu chunk
    for j in range(4):                            # ones, x, x^2, x^3
        head[:, 2 * (1 + j):2 * (1 + j) + 2] = q[j].T
    for k in range(5):                            # relu^3 knots m=6..10
        head[:, 2 * (5 + k):2 * (5 + k) + 2] = wp[:, :, 6 + k].T
    return head.astype(np.float32)


def _fold_weights(inp):
    wrs = inp["w_root_se"] + inp["w_root_ue"]
    wbigT = (wrs @ inp["w_email"]).T.copy()                     # [768, 128]
    mcagg = np.zeros((9, 128), np.float32)
    mcagg[0:8] = (inp["w_rel_ue"] @ inp["w_url"]).T             # url feats
    mcagg[8] = inp["w_rel_se"] @ inp["w_sender"][:, 0]          # sender feat
    # biases fold to a constant vector; this problem has all-zero biases.
    bias = (inp["b_rel_se"] + inp["b_rel_ue"] + wrs @ inp["b_email"])
    assert (np.all(inp["b_sender"] == 0) and np.all(inp["b_url"] == 0)
            and np.all(bias == 0)), "nonzero biases need the count path"
    head = _head_weights(inp["base_w"], inp["spline_w"])
    return wbigT, mcagg, head


def _wrap_idx16(flat):
    """int16 slot list -> [128, n/16] wrapped in 16 partitions, tiled to 128."""
    n = flat.shape[0]
    a = flat.astype(np.int16).reshape(n // 16, 16).T            # [16, n/16]
    return np.tile(a, (8, 1))


def _prep_edges(inp):
    """Flat per-class dst-sorted streams, x64-padded at 512-node pages.

    Per class returns: idx [8][128, E/16] i16; oh [8][128, nblk*512] f8;
    chunks [(idx_off_cols, nidx, pages)] where pages = per-page part lists
    [(local_grp, row_lo, row_hi, blk)] in chunk-local coordinates.
    """
    cls_edges = []
    cls_edges.append((inp["se_src"], inp["se_dst"], NS))                 # S
    ua = inp["ue_src"] < URL_SPLIT
    cls_edges.append((inp["ue_src"][ua], inp["ue_dst"][ua], URL_SPLIT))  # A
    cls_edges.append((inp["ue_src"][~ua] - URL_SPLIT, inp["ue_dst"][~ua],
                      NU - URL_SPLIT))                                   # B

    out = []
    for ci, (src, dst, zrow) in enumerate(cls_edges):
        percore = []
        npage = np.zeros((N_CORES, PAGES), np.int64)
        for c in range(N_CORES):
            sel = (dst >= c * NSH) & (dst < (c + 1) * NSH)
            s, d = src[sel], dst[sel] - c * NSH
            order = np.argsort(d, kind="stable")
            s, d = s[order], d[order]
            percore.append((s, d))
            npage[c] = np.bincount(d // 512, minlength=PAGES)
        e_p = 128 * np.ceil(npage.max(axis=0) / 128).astype(np.int64)
        etot = int(e_p.sum())
        assert etot % 64 == 0
        etot_r = 128 * ((etot + 127) // 128)      # idx stream rounding
        off_p = np.concatenate([[0], np.cumsum(e_p)])

        # chunk structure: consecutive pages, <= CH_MAX edges
        chunks = []           # (p0, p1, off0)
        p0 = 0
        while p0 < PAGES:
            p1 = p0 + 1
            n = int(e_p[p0])
            while p1 < PAGES and n + e_p[p1] <= CH_MAX:
                n += int(e_p[p1]); p1 += 1
            chunks.append((p0, p1, int(off_p[p0]), n))
            p0 = p1
        # per-chunk page part lists (chunk-local rows) + global block ids
        nblk = 0
        chunk_meta = []
        for (cp0, cp1, off0, n) in chunks:
            pages = []
            for p in range(cp0, cp1):
                parts = []
                pos = int(off_p[p]) - off0
                end = pos + int(e_p[p])
                while pos < end:
                    g, lo = pos // 128, pos % 128
                    hi = min(128, lo + (end - pos))
                    parts.append((g, lo, hi, nblk))
                    nblk += 1
                    pos += hi - lo
                pages.append(tuple(parts))
            nidx = 128 * ((n + 127) // 128) if (cp1 == PAGES) else n
            # gather nidx must cover the chunk; trailing idx pad with zrow
            chunk_meta.append((off0 // 16, n, tuple(pages)))

        idxs, ohs = [], []
        for c in range(N_CORES):
            s, d = percore[c]
            slots = np.full(etot_r, zrow, np.int32)
            pagecol = np.full(etot_r, -1, np.int64)
            pstart = np.concatenate([[0], np.cumsum(npage[c])])
            for p in range(PAGES):
                a, b = pstart[p], pstart[p + 1]
                o = off_p[p]
                slots[o:o + (b - a)] = s[a:b]
                pagecol[o:o + (b - a)] = d[a:b] - 512 * p
            idxs.append(_wrap_idx16(slots))
            oh = np.zeros((128, nblk * 512), F8)
            for (ioff, n, pages) in chunk_meta:
                off0 = ioff * 16
                for parts in pages:
                    for (g, lo, hi, blk) in parts:
                        for row in range(lo, hi):
                            e = off0 + g * 128 + row
                            col = pagecol[e]
                            if col >= 0:
                                oh[row, blk * 512 + col] = 1
            ohs.append(oh)
        out.append(dict(idx=idxs, oh=ohs, zrow=zrow, etot=etot_r, nblk=nblk,
                        chunks=tuple(chunk_meta)))
    return out


# ----------------------------------------------------------------- device build
def _build(meta):
    """meta: per class (etot, nblk, chunks); cross-core static."""
    nc = bacc.Bacc("TRN2", target_bir_lowering=False, debug=False,
                   num_devices=N_CORES)
    dt = lambda n, s, d, k: nc.dram_tensor(n, s, d, kind=k).ap()
    xT = dt("xT", [KIN, NP], BF16, "ExternalInput")
    tabs, idxd, ohd = [], [], []
    nrows = (NS + 1, URL_SPLIT + 1, NU - URL_SPLIT + 1)
    for ci in range(3):
        etot, nblk = meta[ci][0], meta[ci][1]
        tabs.append(dt(f"tab{ci}", [nrows[ci], ELEM], BF16, "ExternalInput"))
        idxd.append(dt(f"idx{ci}", [128, etot // 16], I16, "ExternalInput"))
        ohd.append(dt(f"oh{ci}", [128, nblk * 512], FP8, "ExternalInput"))
    wbigT = dt("wbigT", [KIN, HID], BF16, "ExternalInput")
    mcagg = dt("mcagg", [9, HID], BF16, "ExternalInput")
    whead = dt("whead", [HID, 20], F32, "ExternalInput")
    whq0 = dt("whq0", [2, 1], F32, "ExternalInput")
    whsil = dt("whsil", [HID, 2], F16, "ExternalInput")
    outT = dt("outT", [2, NP], F32, "ExternalOutput")

    MAXG = CH_MAX // 128 + 2
    with tile.TileContext(nc) as tc:
        import contextlib
        with contextlib.ExitStack() as ctx:
            persist = ctx.enter_context(tc.tile_pool(name="persist", bufs=1))
            gpool = ctx.enter_context(tc.tile_pool(name="gath", bufs=2))
            opool = ctx.enter_context(tc.tile_pool(name="oh", bufs=1))
            xpool = ctx.enter_context(tc.tile_pool(name="x", bufs=2))
            ew = ctx.enter_context(tc.tile_pool(name="ew", bufs=2))
            psA = ctx.enter_context(tc.tile_pool(name="psA", bufs=2,
                                                 space="PSUM"))
            psB = ctx.enter_context(tc.tile_pool(name="psB", bufs=2,
                                                 space="PSUM"))
            psO = ctx.enter_context(tc.tile_pool(name="psO", bufs=2,
                                                 space="PSUM"))

            # ---- persistent small tensors
            wb = persist.tile([128, NKC * HID], BF16)
            nc.sync.dma_start(
                out=wb[:].rearrange("p (c h) -> p c h", c=NKC),
                in_=wbigT.rearrange("(c p) h -> p c h", p=128))
            mcA = persist.tile([9, HID], BF16)
            nc.sync.dma_start(out=mcA[:], in_=mcagg)
            wh = persist.tile([HID, 20], F32)
            nc.sync.dma_start(out=wh[:], in_=whead)
            wq0 = persist.tile([2, 1], F32)
            nc.sync.dma_start(out=wq0[:], in_=whq0)
            whs = persist.tile([HID, 2], F16)
            nc.sync.dma_start(out=whs[:], in_=whsil)
            kbias = persist.tile([128, 5], F32)
            for k, tk in enumerate(KNOTS):
                nc.vector.memset(kbias[:, k:k + 1], -tk)

            # ---- phase B (per 512-node page)
            def phase_b(p, pg):
                ns = slice(p * 512, (p + 1) * 512)
                xs = xpool.tile([128, NKC * 512], BF16, tag="xs")
                nc.sync.dma_start(
                    out=xs[:].rearrange("q (c n) -> q c n", c=NKC),
                    in_=xT[:, ns].rearrange("(c q) n -> q c n", q=128))
                pP = psB.tile([128, 512], F32, space="PSUM", tag="pP")
                for k in range(NKC):
                    nc.tensor.matmul(
                        out=pP[:], lhsT=wb[:, k * HID:(k + 1) * HID],
                        rhs=xs[:, k * 512:(k + 1) * 512],
                        start=(k == 0), stop=False)
                nc.tensor.matmul(out=pP[:], lhsT=mcA[:], rhs=pg[0:9, :],
                                 start=False, stop=True)

                # KAN head; f32 chunks. DVE ops 1-port only.
                xt = ew.tile([128, 512], F32, tag="xt")     # clamp(h,0,2.2)
                nc.vector.tensor_scalar(out=xt[:], in0=pP[:], scalar1=0.0,
                                        scalar2=XCLAMP,
                                        op0=mybir.AluOpType.max,
                                        op1=mybir.AluOpType.min)
                sil = ew.tile([128, 512], F32, tag="sil")
                nc.scalar.activation(sil[:], pP[:],
                                     mybir.ActivationFunctionType.Silu)
                rsil = ew.tile([128, 512], F16, tag="rsil")
                nc.scalar.activation(rsil[:], sil[:],
                                     mybir.ActivationFunctionType.Relu)
                x2 = ew.tile([128, 512], F32, tag="x2")
                nc.scalar.activation(x2[:], xt[:],
                                     mybir.ActivationFunctionType.Square)
                x3 = ew.tile([128, 512], F32, tag="x3")
                nc.vector.tensor_tensor(out=x3[:], in0=x2[:], in1=xt[:],
                                        op=mybir.AluOpType.mult)
                pO = psO.tile([2, 512], F32, space="PSUM", tag="pO")
                nc.tensor.matmul(out=pO[:], lhsT=whs[:], rhs=rsil[:],
                                 start=True, stop=False)
                for j, ck in ((2, xt), (3, x2), (4, x3)):
                    nc.tensor.matmul(out=pO[:], lhsT=wh[:, 2 * j:2 * j + 2],
                                     rhs=ck[:], start=False, stop=False)
                for k in range(5):
                    r = ew.tile([128, 512], F32, tag="r")
                    nc.scalar.activation(r[:], xt[:],
                                         mybir.ActivationFunctionType.Relu,
                                         bias=kbias[:, k:k + 1])
                    r2 = ew.tile([128, 512], F32, tag="r2")
                    nc.vector.tensor_tensor(out=r2[:], in0=r[:], in1=r[:],
                                            op=mybir.AluOpType.mult)
                    r3 = ew.tile([128, 512], F32, tag="r3")
                    nc.vector.tensor_tensor(out=r3[:], in0=r2[:], in1=r[:],
                                            op=mybir.AluOpType.mult)
                    nc.tensor.matmul(out=pO[:],
                                     lhsT=wh[:, 10 + 2 * k:12 + 2 * k],
                                     rhs=r3[:], start=False, stop=(k == 4))
                ot = ew.tile([2, 512], F32, tag="ot")
                nc.vector.tensor_scalar_add(out=ot[:], in0=pO[:],
                                            scalar1=wq0[:])
                nc.sync.dma_start(out=outT[:, ns], in_=ot[:])

            # ---- main loop: gathers stream in page-aligned chunks;
            #      per page: scatter part-matmuls -> psum -> phase B.
            cptr = [0, 0, 0]
            cur = [None, None, None]        # (gt, oh, pages, next_page_idx)
            nextp = [0, 0, 0]               # first page of next chunk
            for p in range(PAGES):
                for ci in range(3):
                    etot, nblk, chunks = meta[ci]
                    if nextp[ci] == p:
                        ioff, n, pages = chunks[cptr[ci]]
                        ng = (n + 127) // 128
                        ncols = (n + 15) // 16
                        isb = gpool.tile([128, CH_MAX // 16 + 8], I16,
                                         tag=f"i{ci}")
                        nc.sync.dma_start(
                            out=isb[:, :ncols],
                            in_=idxd[ci][:, ioff:ioff + ncols])
                        gt = gpool.tile([128, MAXG, ELEM], BF16, tag=f"g{ci}")
                        if n % 128:
                            nc.vector.memset(gt[:, ng - 1, :], 0.0)
                        nc.gpsimd.dma_gather(
                            out_ap=gt[:, :ng, :], in_ap=tabs[ci],
                            idxs_ap=isb[:, :ncols],
                            num_idxs=n, num_idxs_reg=n, elem_size=ELEM,
                            single_packet=False)
                        blk0 = pages[0][0][3]
                        nb = sum(len(pp) for pp in pages)
                        oh = opool.tile([128, (MAXG + 8) * 512], FP8,
                                        tag=f"o{ci}")
                        nc.sync.dma_start(
                            out=oh[:, :nb * 512],
                            in_=ohd[ci][:, blk0 * 512:(blk0 + nb) * 512])
                        cur[ci] = (gt, oh, pages, blk0)
                        cptr[ci] += 1
                        nextp[ci] = p + len(pages)
                pg = psA.tile([16, 512], F32, space="PSUM", tag="pg")
                mms = []
                for ci in range(3):
                    gt, oh, pages, blk0 = cur[ci]
                    pidx = p - (nextp[ci] - len(pages))
                    for (g, lo, hi, blk) in pages[pidx]:
                        mms.append((gt, oh, g, lo, hi, blk - blk0))
                for i, (gt, oh, g, lo, hi, blk) in enumerate(mms):
                    nc.tensor.matmul(
                        out=pg[0:9, :], lhsT=gt[:, g, 0:9],
                        rhs=oh[:, blk * 512:(blk + 1) * 512],
                        start=(i == 0), stop=(i == len(mms) - 1))
                pgs = ew.tile([9, 512], BF16, tag="pgs")
                nc.scalar.copy(out=pgs[:], in_=pg[0:9, :])
                phase_b(p, pgs)

    nc.compile()
    return nc


# ----------------------------------------------------------------- entry point
def kernel(**inp):
    inp = {k: np.asarray(v) for k, v in inp.items()}
    wbigT, mcagg, head = _fold_weights(inp)
    eprep = _prep_edges(inp)

    meta = tuple((e["etot"], e["nblk"], e["chunks"]) for e in eprep)
    if meta not in _CACHE:
        _CACHE[meta] = _build(meta)
    nc = _CACHE[meta]

    tabS = np.zeros((NS + 1, ELEM), BF)
    tabS[:NS, 8] = inp["x_sender"][:, 0].astype(BF)
    tabA = np.zeros((URL_SPLIT + 1, ELEM), BF)
    tabA[:URL_SPLIT, 0:8] = inp["x_url"][:URL_SPLIT].astype(BF)
    tabB = np.zeros((NU - URL_SPLIT + 1, ELEM), BF)
    tabB[:NU - URL_SPLIT, 0:8] = inp["x_url"][URL_SPLIT:].astype(BF)
    q0 = head[:, 2:4].sum(axis=0).reshape(2, 1)

    in_maps = []
    for c in range(N_CORES):
        xsh = np.zeros((KIN, NP), BF)
        xsh[:, :NSH] = inp["x_email"][c * NSH:(c + 1) * NSH].T.astype(BF)
        m = {"xT": xsh, "tab0": tabS, "tab1": tabA, "tab2": tabB,
             "wbigT": wbigT.astype(BF), "mcagg": mcagg.astype(BF),
             "whead": head, "whq0": q0,
             "whsil": head[:, 0:2].astype(np.float16)}
        for ci in range(3):
            m[f"idx{ci}"] = eprep[ci]["idx"][c]
            m[f"oh{ci}"] = eprep[ci]["oh"][c]
        in_maps.append(m)

    global _LAST_RESULT
    trace = os.environ.get("KERNEL_TRACE", "0") == "1"
    res = run_bass_kernel_spmd(nc, in_maps, core_ids=list(range(N_CORES)),
                               trace=trace)
    _LAST_RESULT = res
    out = np.empty((NE, 2), np.float32)
    for c in range(N_CORES):
        out[c * NSH:(c + 1) * NSH] = res.results[c]["outT"][:, :NSH].T
    return out


# revision 10
# speedup vs baseline: 1.0443x; 1.0443x over previous
"""Trainium2 Bass kernel for nn_HKANGNN (hetero GraphConv + KAN head).

Math (only the email-node output path matters):
  e    = x_email @ w_email.T + b_email
  agg_se[n] = sum_{se edges -> n} (x_sender[src] @ w_sender.T + b_sender)
  agg_ue[n] = sum_{ue edges -> n} (x_url[src]    @ w_url.T    + b_url)
  out_e = agg_se @ w_rel_se.T + b_rel_se + agg_ue @ w_rel_ue.T + b_rel_ue
        + e @ (w_root_se + w_root_ue).T
  h = relu(out_e);  out = silu(h) @ base_w.T + einsum(b_splines(h), spline_w)

Device strategy (8 cores, email nodes sharded 12500/core, padded to 12800):
  * by linearity the per-edge payload is the RAW source features; the tiny
    projections fold into mcAgg on host (biases are zero -> no count terms).
  * edges per class (sender / urlA / urlB) are dst-sorted into a FLAT stream,
    padded to x128 at 512-node page boundaries (~20% pad).
  * dma_gather fetches 256B rows per edge (Q7-descriptor-bound ~8ns/edge);
    scatter is one fp8 one-hot matmul [K<=128 x 512n] per part (page-run of a
    128-edge group) into a per-page PSUM accumulator; the chunk/page/part
    schedule depends only on cross-core maxima -> SPMD-uniform.
  * projection: out_e.T accumulated in PSUM over 6 K-chunks of
    (Wrootsum@w_email).T (bf16) + mcAgg @ page.
  * KAN head: spline(h) == q0 + q1 x + q2 x^2 + q3 x^3 + sum_k W'_k relu(x-t_k)^3
    with x = clamp(h,0,2.2); chunks stay f32 (power-basis coefficients amplify
    2-byte rounding); knot relus on the scalar engine, cubes as 1-port DVE
    tensor_tensor (avoids SWDGE/DVE 2-port contention); q0 added in the
    output copy.
"""

import os
import numpy as np
import ml_dtypes

import concourse.bass as bass
import concourse.mybir as mybir
import concourse.tile as tile
from concourse import bacc
from concourse.bass_utils import run_bass_kernel_spmd

F32 = mybir.dt.float32
F16 = mybir.dt.float16
BF16 = mybir.dt.bfloat16
FP8 = mybir.dt.float8e4
I16 = mybir.dt.int16
BF = ml_dtypes.bfloat16
F8 = ml_dtypes.float8_e4m3

N_CORES = 8
HID = 128
NE, NS, NU = 100000, 30000, 50000
NSH = NE // N_CORES          # 12500 real nodes per core
NP = 12800                   # padded: 25 pages x 512 nodes
PAGES = NP // 512
KIN = 768
NKC = KIN // 128             # 6 projection K-chunks
URL_SPLIT = 25600            # url class A rows [0,25600), B rows [25600,50000)
ELEM = 128                   # gather row: 128 bf16 = 256 B
CH_MAX = 4096                # max edges per dma_gather call
KNOTS = (0.2, 0.6, 1.0, 1.4, 1.8)
XCLAMP = 2.2

_LAST_RESULT = None
_CACHE = {}


# ----------------------------------------------------------------- host folds
def _head_weights(base_w, spline_w):
    """[128, 20] f32: lhsT ([d,2]) per head chunk, order
    [silu, ones, x, x^2, x^3, R(.2)^3, R(.6)^3, R(1.0)^3, R(1.4)^3, R(1.8)^3]."""
    c = np.array([1.0, -4.0, 6.0, -4.0, 1.0], np.float64)
    h = 0.4
    scale = 1.0 / (6.0 * h ** 3)
    O, D, B = spline_w.shape                      # [2, 128, 8]
    wp = np.zeros((O, D, 11), np.float64)         # W'[o,d,m], m=0..10
    for m in range(11):
        for j in range(5):
            b = m - j
            if 0 <= b < B:
                wp[:, :, m] += spline_w[:, :, b].astype(np.float64) * c[j] * scale
    t = np.arange(11) * h - 2.2                   # knot m at t_m
    q = np.zeros((4, O, D), np.float64)           # poly coeffs from m=0..5
    for m in range(6):
        q[0] += -t[m] ** 3 * wp[:, :, m]
        q[1] += 3 * t[m] ** 2 * wp[:, :, m]
        q[2] += -3 * t[m] * wp[:, :, m]
        q[3] += wp[:, :, m]
    head = np.zeros((D, 20), np.float64)
    head[:, 0:2] = base_w.T                       # silu chunk
    for j in range(4):                            # ones, x, x^2, x^3
        head[:, 2 * (1 + j):2 * (1 + j) + 2] = q[j].T
    for k in range(5):                            # relu^3 knots m=6..10
        head[:, 2 * (5 + k):2 * (5 + k) + 2] = wp[:, :, 6 + k].T
    return head.astype(np.float32)


def _fold_weights(inp):
    wrs = inp["w_root_se"] + inp["w_root_ue"]
    wbigT = (wrs @ inp["w_email"]).T.copy()                     # [768, 128]
    mcagg = np.zeros((9, 128), np.float32)
    mcagg[0:8] = (inp["w_rel_ue"] @ inp["w_url"]).T             # url feats
    mcagg[8] = inp["w_rel_se"] @ inp["w_sender"][:, 0]          # sender feat
    # biases fold to a constant vector; this problem has all-zero biases.
    bias = (inp["b_rel_se"] + inp["b_rel_ue"] + wrs @ inp["b_email"])
    assert (np.all(inp["b_sender"] == 0) and np.all(inp["b_url"] == 0)
            and np.all(bias == 0)), "nonzero biases need the count path"
    head = _head_weights(inp["base_w"], inp["spline_w"])
    return wbigT, mcagg, head


def _wrap_idx16(flat):
    """int16 slot list -> [128, n/16] wrapped in 16 partitions, tiled to 128."""
    n = flat.shape[0]
    a = flat.astype(np.int16).reshape(n // 16, 16).T            # [16, n/16]
    return np.tile(a, (8, 1))


def _prep_edges(inp):
    """Flat per-class dst-sorted streams, x64-padded at 512-node pages.

    Per class returns: idx [8][128, E/16] i16; oh [8][128, nblk*512] f8;
    chunks [(idx_off_cols, nidx, pages)] where pages = per-page part lists
    [(local_grp, row_lo, row_hi, blk)] in chunk-local coordinates.
    """
    cls_edges = []
    cls_edges.append((inp["se_src"], inp["se_dst"], NS))                 # S
    ua = inp["ue_src"] < URL_SPLIT
    cls_edges.append((inp["ue_src"][ua], inp["ue_dst"][ua], URL_SPLIT))  # A
    cls_edges.append((inp["ue_src"][~ua] - URL_SPLIT, inp["ue_dst"][~ua],
                      NU - URL_SPLIT))                                   # B

    out = []
    for ci, (src, dst, zrow) in enumerate(cls_edges):
        percore = []
        npage = np.zeros((N_CORES, PAGES), np.int64)
        for c in range(N_CORES):
            sel = (dst >= c * NSH) & (dst < (c + 1) * NSH)
            s, d = src[sel], dst[sel] - c * NSH
            order = np.argsort(d, kind="stable")
            s, d = s[order], d[order]
            percore.append((s, d))
            npage[c] = np.bincount(d // 512, minlength=PAGES)
        e_p = 64 * np.ceil(npage.max(axis=0) / 64).astype(np.int64)
        etot = int(e_p.sum())
        assert etot % 64 == 0
        etot_r = 128 * ((etot + 127) // 128)      # idx stream rounding
        off_p = np.concatenate([[0], np.cumsum(e_p)])

        # chunk structure: consecutive pages, <= CH_MAX edges
        chunks = []           # (p0, p1, off0)
        p0 = 0
        while p0 < PAGES:
            p1 = p0 + 1
            n = int(e_p[p0])
            while p1 < PAGES and n + e_p[p1] <= CH_MAX:
                n += int(e_p[p1]); p1 += 1
            chunks.append((p0, p1, int(off_p[p0]), n))
            p0 = p1
        # per-chunk page part lists (chunk-local rows) + global block ids
        nblk = 0
        chunk_meta = []
        for (cp0, cp1, off0, n) in chunks:
            pages = []
            for p in range(cp0, cp1):
                parts = []
                pos = int(off_p[p]) - off0
                end = pos + int(e_p[p])
                while pos < end:
                    g, lo = pos // 128, pos % 128
                    hi = min(128, lo + (end - pos))
                    parts.append((g, lo, hi, nblk))
                    nblk += 1
                    pos += hi - lo
                pages.append(tuple(parts))
            nidx = 128 * ((n + 127) // 128) if (cp1 == PAGES) else n
            # gather nidx must cover the chunk; trailing idx pad with zrow
            chunk_meta.append((off0 // 16, n, tuple(pages)))

        idxs, ohs = [], []
        for c in range(N_CORES):
            s, d = percore[c]
            slots = np.full(etot_r, zrow, np.int32)
            pagecol = np.full(etot_r, -1, np.int64)
            pstart = np.concatenate([[0], np.cumsum(npage[c])])
            for p in range(PAGES):
                a, b = pstart[p], pstart[p + 1]
                o = off_p[p]
                slots[o:o + (b - a)] = s[a:b]
                pagecol[o:o + (b - a)] = d[a:b] - 512 * p
            idxs.append(_wrap_idx16(slots))
            oh = np.zeros((128, nblk * 512), F8)
            for (ioff, n, pages) in chunk_meta:
                off0 = ioff * 16
                for parts in pages:
                    for (g, lo, hi, blk) in parts:
                        for row in range(lo, hi):
                            e = off0 + g * 128 + row
                            col = pagecol[e]
                            if col >= 0:
                                oh[row, blk * 512 + col] = 1
            ohs.append(oh)
        out.append(dict(idx=idxs, oh=ohs, zrow=zrow, etot=etot_r, nblk=nblk,
                        chunks=tuple(chunk_meta)))
    return out


# ----------------------------------------------------------------- device build
def _build(meta):
    """meta: per class (etot, nblk, chunks); cross-core static."""
    nc = bacc.Bacc("TRN2", target_bir_lowering=False, debug=False,
                   num_devices=N_CORES)
    dt = lambda n, s, d, k: nc.dram_tensor(n, s, d, kind=k).ap()
    xT = dt("xT", [KIN, NP], BF16, "ExternalInput")
    tabs, idxd, ohd = [], [], []
    nrows = (NS + 1, URL_SPLIT + 1, NU - URL_SPLIT + 1)
    for ci in range(3):
        etot, nblk = meta[ci][0], meta[ci][1]
        tabs.append(dt(f"tab{ci}", [nrows[ci], ELEM], BF16, "ExternalInput"))
        idxd.append(dt(f"idx{ci}", [128, etot // 16], I16, "ExternalInput"))
        ohd.append(dt(f"oh{ci}", [128, nblk * 512], FP8, "ExternalInput"))
    wbigT = dt("wbigT", [KIN, HID], BF16, "ExternalInput")
    mcagg = dt("mcagg", [9, HID], BF16, "ExternalInput")
    whead = dt("whead", [HID, 20], F32, "ExternalInput")
    whq0 = dt("whq0", [2, 1], F32, "ExternalInput")
    whsil = dt("whsil", [HID, 2], F16, "ExternalInput")
    outT = dt("outT", [2, NP], F32, "ExternalOutput")

    MAXG = CH_MAX // 128 + 2
    with tile.TileContext(nc) as tc:
        import contextlib
        with contextlib.ExitStack() as ctx:
            persist = ctx.enter_context(tc.tile_pool(name="persist", bufs=1))
            gpool = ctx.enter_context(tc.tile_pool(name="gath", bufs=2))
            opool = ctx.enter_context(tc.tile_pool(name="oh", bufs=1))
            xpool = ctx.enter_context(tc.tile_pool(name="x", bufs=2))
            ew = ctx.enter_context(tc.tile_pool(name="ew", bufs=2))
            psA = ctx.enter_context(tc.tile_pool(name="psA", bufs=2,
                                                 space="PSUM"))
            psB = ctx.enter_context(tc.tile_pool(name="psB", bufs=2,
                                                 space="PSUM"))
            psO = ctx.enter_context(tc.tile_pool(name="psO", bufs=2,
                                                 space="PSUM"))

            # ---- persistent small tensors
            wb = persist.tile([128, NKC * HID], BF16)
            nc.sync.dma_start(
                out=wb[:].rearrange("p (c h) -> p c h", c=NKC),
                in_=wbigT.rearrange("(c p) h -> p c h", p=128))
            mcA = persist.tile([9, HID], BF16)
            nc.sync.dma_start(out=mcA[:], in_=mcagg)
            wh = persist.tile([HID, 20], F32)
            nc.sync.dma_start(out=wh[:], in_=whead)
            wq0 = persist.tile([2, 1], F32)
            nc.sync.dma_start(out=wq0[:], in_=whq0)
            whs = persist.tile([HID, 2], F16)
            nc.sync.dma_start(out=whs[:], in_=whsil)
            kbias = persist.tile([128, 5], F32)
            for k, tk in enumerate(KNOTS):
                nc.vector.memset(kbias[:, k:k + 1], -tk)

            # ---- phase B (per 512-node page)
            def phase_b(p, pg):
                ns = slice(p * 512, (p + 1) * 512)
                xs = xpool.tile([128, NKC * 512], BF16, tag="xs")
                nc.sync.dma_start(
                    out=xs[:].rearrange("q (c n) -> q c n", c=NKC),
                    in_=xT[:, ns].rearrange("(c q) n -> q c n", q=128))
                pP = psB.tile([128, 512], F32, space="PSUM", tag="pP")
                for k in range(NKC):
                    nc.tensor.matmul(
                        out=pP[:], lhsT=wb[:, k * HID:(k + 1) * HID],
                        rhs=xs[:, k * 512:(k + 1) * 512],
                        start=(k == 0), stop=False)
                nc.tensor.matmul(out=pP[:], lhsT=mcA[:], rhs=pg[0:9, :],
                                 start=False, stop=True)

                # KAN head; f32 chunks. DVE ops 1-port only.
                xt = ew.tile([128, 512], F32, tag="xt")     # clamp(h,0,2.2)
                nc.vector.tensor_scalar(out=xt[:], in0=pP[:], scalar1=0.0,
                                        scalar2=XCLAMP,
                                        op0=mybir.AluOpType.max,
                                        op1=mybir.AluOpType.min)
                sil = ew.tile([128, 512], F32, tag="sil")
                nc.scalar.activation(sil[:], pP[:],
                                     mybir.ActivationFunctionType.Silu)
                rsil = ew.tile([128, 512], F16, tag="rsil")
                nc.scalar.activation(rsil[:], sil[:],
                                     mybir.ActivationFunctionType.Relu)
                x2 = ew.tile([128, 512], F32, tag="x2")
                nc.scalar.activation(x2[:], xt[:],
                                     mybir.ActivationFunctionType.Square)
                x3 = ew.tile([128, 512], F32, tag="x3")
                nc.vector.tensor_tensor(out=x3[:], in0=x2[:], in1=xt[:],
                                        op=mybir.AluOpType.mult)
                pO = psO.tile([2, 512], F32, space="PSUM", tag="pO")
                nc.tensor.matmul(out=pO[:], lhsT=whs[:], rhs=rsil[:],
                                 start=True, stop=False)
                for j, ck in ((2, xt), (3, x2), (4, x3)):
                    nc.tensor.matmul(out=pO[:], lhsT=wh[:, 2 * j:2 * j + 2],
                                     rhs=ck[:], start=False, stop=False)
                for k in range(5):
                    r = ew.tile([128, 512], F32, tag="r")
                    nc.scalar.activation(r[:], xt[:],
                                         mybir.ActivationFunctionType.Relu,
                                         bias=kbias[:, k:k + 1])
                    r2 = ew.tile([128, 512], F32, tag="r2")
                    nc.vector.tensor_tensor(out=r2[:], in0=r[:], in1=r[:],
                                            op=mybir.AluOpType.mult)
                    r3 = ew.tile([128, 512], F32, tag="r3")
                    nc.vector.tensor_tensor(out=r3[:], in0=r2[:], in1=r[:],
                                            op=mybir.AluOpType.mult)
                    nc.tensor.matmul(out=pO[:],
                                     lhsT=wh[:, 10 + 2 * k:12 + 2 * k],
                                     rhs=r3[:], start=False, stop=(k == 4))
                ot = ew.tile([2, 512], F32, tag="ot")
                nc.vector.tensor_scalar_add(out=ot[:], in0=pO[:],
                                            scalar1=wq0[:])
                nc.sync.dma_start(out=outT[:, ns], in_=ot[:])

            # ---- main loop: gathers stream in page-aligned chunks;
            #      per page: scatter part-matmuls -> psum -> phase B.
            cptr = [0, 0, 0]
            cur = [None, None, None]        # (gt, oh, pages, next_page_idx)
            nextp = [0, 0, 0]               # first page of next chunk
            for p in range(PAGES):
                for ci in range(3):
                    etot, nblk, chunks = meta[ci]
                    if nextp[ci] == p:
                        ioff, n, pages = chunks[cptr[ci]]
                        ng = (n + 127) // 128
                        ncols = (n + 15) // 16
                        isb = gpool.tile([128, CH_MAX // 16 + 8], I16,
                                         tag=f"i{ci}")
                        nc.sync.dma_start(
                            out=isb[:, :ncols],
                            in_=idxd[ci][:, ioff:ioff + ncols])
                        gt = gpool.tile([128, MAXG, ELEM], BF16, tag=f"g{ci}")
                        if n % 128:
                            nc.vector.memset(gt[:, ng - 1, :], 0.0)
                        nc.gpsimd.dma_gather(
                            out_ap=gt[:, :ng, :], in_ap=tabs[ci],
                            idxs_ap=isb[:, :ncols],
                            num_idxs=n, num_idxs_reg=n, elem_size=ELEM,
                            single_packet=False)
                        blk0 = pages[0][0][3]
                        nb = sum(len(pp) for pp in pages)
                        oh = opool.tile([128, (MAXG + 8) * 512], FP8,
                                        tag=f"o{ci}")
                        nc.sync.dma_start(
                            out=oh[:, :nb * 512],
                            in_=ohd[ci][:, blk0 * 512:(blk0 + nb) * 512])
                        cur[ci] = (gt, oh, pages, blk0)
                        cptr[ci] += 1
                        nextp[ci] = p + len(pages)
                pg = psA.tile([16, 512], F32, space="PSUM", tag="pg")
                mms = []
                for ci in range(3):
                    gt, oh, pages, blk0 = cur[ci]
                    pidx = p - (nextp[ci] - len(pages))
                    for (g, lo, hi, blk) in pages[pidx]:
                        mms.append((gt, oh, g, lo, hi, blk - blk0))
                for i, (gt, oh, g, lo, hi, blk) in enumerate(mms):
                    nc.tensor.matmul(
                        out=pg[0:9, :], lhsT=gt[:, g, 0:9],
                        rhs=oh[:, blk * 512:(blk + 1) * 512],
                        start=(i == 0), stop=(i == len(mms) - 1))
                pgs = ew.tile([9, 512], BF16, tag="pgs")
                nc.scalar.copy(out=pgs[:], in_=pg[0:9, :])
                phase_b(p, pgs)

    nc.compile()
    return nc


# ----------------------------------------------------------------- entry point
def kernel(**inp):
    inp = {k: np.asarray(v) for k, v in inp.items()}
    wbigT, mcagg, head = _fold_weights(inp)
    eprep = _prep_edges(inp)

    meta = tuple((e["etot"], e["nblk"], e["chunks"]) for e in eprep)
    if meta not in _CACHE:
        _CACHE[meta] = _build(meta)
    nc = _CACHE[meta]

    tabS = np.zeros((NS + 1, ELEM), BF)
    tabS[:NS, 8] = inp["x_sender"][:, 0].astype(BF)
    tabA = np.zeros((URL_SPLIT + 1, ELEM), BF)
    tabA[:URL_SPLIT, 0:8] = inp["x_url"][:URL_SPLIT].astype(BF)
    tabB = np.zeros((NU - URL_SPLIT + 1, ELEM), BF)
    tabB[:NU - URL_SPLIT, 0:8] = inp["x_url"][URL_SPLIT:].astype(BF)
    q0 = head[:, 2:4].sum(axis=0).reshape(2, 1)

    in_maps = []
    for c in range(N_CORES):
        xsh = np.zeros((KIN, NP), BF)
        xsh[:, :NSH] = inp["x_email"][c * NSH:(c + 1) * NSH].T.astype(BF)
        m = {"xT": xsh, "tab0": tabS, "tab1": tabA, "tab2": tabB,
             "wbigT": wbigT.astype(BF), "mcagg": mcagg.astype(BF),
             "whead": head, "whq0": q0,
             "whsil": head[:, 0:2].astype(np.float16)}
        for ci in range(3):
            m[f"idx{ci}"] = eprep[ci]["idx"][c]
            m[f"oh{ci}"] = eprep[ci]["oh"][c]
        in_maps.append(m)

    global _LAST_RESULT
    trace = os.environ.get("KERNEL_TRACE", "0") == "1"
    res = run_bass_kernel_spmd(nc, in_maps, core_ids=list(range(N_CORES)),
                               trace=trace)
    _LAST_RESULT = res
    out = np.empty((NE, 2), np.float32)
    for c in range(N_CORES):
        out[c * NSH:(c + 1) * NSH] = res.results[c]["outT"][:, :NSH].T
    return out


# revision 13
# speedup vs baseline: 1.0856x; 1.0396x over previous
"""Trainium2 Bass kernel for nn_HKANGNN (hetero GraphConv + KAN head).

Math (only the email-node output path matters):
  e    = x_email @ w_email.T + b_email
  agg_se[n] = sum_{se edges -> n} (x_sender[src] @ w_sender.T + b_sender)
  agg_ue[n] = sum_{ue edges -> n} (x_url[src]    @ w_url.T    + b_url)
  out_e = agg_se @ w_rel_se.T + b_rel_se + agg_ue @ w_rel_ue.T + b_rel_ue
        + e @ (w_root_se + w_root_ue).T
  h = relu(out_e);  out = silu(h) @ base_w.T + einsum(b_splines(h), spline_w)

Device strategy (8 cores, email nodes sharded 12500/core, padded to 12800):
  * by linearity the per-edge payload is the RAW source features; the tiny
    projections fold into mcAgg on host (biases are zero -> no count terms).
  * edges per class (sender / urlA / urlB) are dst-sorted into a FLAT stream,
    padded to x128 at 512-node page boundaries (~20% pad).
  * dma_gather fetches 256B rows per edge (Q7-descriptor-bound ~8ns/edge);
    scatter is one fp8 one-hot matmul [K<=128 x 512n] per part (page-run of a
    128-edge group) into a per-page PSUM accumulator; the chunk/page/part
    schedule depends only on cross-core maxima -> SPMD-uniform.
  * projection: out_e.T accumulated in PSUM over 6 K-chunks of
    (Wrootsum@w_email).T (bf16) + mcAgg @ page.
  * KAN head: spline(h) == q0 + q1 x + q2 x^2 + q3 x^3 + sum_k W'_k relu(x-t_k)^3
    with x = clamp(h,0,2.2); chunks stay f32 (power-basis coefficients amplify
    2-byte rounding); knot relus on the scalar engine, cubes as 1-port DVE
    tensor_tensor (avoids SWDGE/DVE 2-port contention); q0 added in the
    output copy.
"""

import os
import numpy as np
import ml_dtypes

import concourse.bass as bass
import concourse.mybir as mybir
import concourse.tile as tile
from concourse import bacc
from concourse.bass_utils import run_bass_kernel_spmd

F32 = mybir.dt.float32
F16 = mybir.dt.float16
BF16 = mybir.dt.bfloat16
FP8 = mybir.dt.float8e4
I16 = mybir.dt.int16
BF = ml_dtypes.bfloat16
F8 = ml_dtypes.float8_e4m3

N_CORES = 8
HID = 128
NE, NS, NU = 100000, 30000, 50000
NSH = NE // N_CORES          # 12500 real nodes per core
NP = 12800                   # padded: 25 pages x 512 nodes
PAGES = NP // 512
KIN = 768
NKC = KIN // 128             # 6 projection K-chunks
URL_SPLIT = 25600            # url class A rows [0,25600), B rows [25600,50000)
ELEM = 128                   # gather row: 128 bf16 = 256 B
CH_MAX = 4096                # max edges per dma_gather call
KNOTS = (0.2, 0.6, 1.0, 1.4, 1.8)
XCLAMP = 2.2

_LAST_RESULT = None
_CACHE = {}


# ----------------------------------------------------------------- host folds
def _head_weights(base_w, spline_w):
    """[128, 20] f32: lhsT ([d,2]) per head chunk, order
    [silu, ones, x, x^2, x^3, R(.2)^3, R(.6)^3, R(1.0)^3, R(1.4)^3, R(1.8)^3]."""
    c = np.array([1.0, -4.0, 6.0, -4.0, 1.0], np.float64)
    h = 0.4
    scale = 1.0 / (6.0 * h ** 3)
    O, D, B = spline_w.shape                      # [2, 128, 8]
    wp = np.zeros((O, D, 11), np.float64)         # W'[o,d,m], m=0..10
    for m in range(11):
        for j in range(5):
            b = m - j
            if 0 <= b < B:
                wp[:, :, m] += spline_w[:, :, b].astype(np.float64) * c[j] * scale
    t = np.arange(11) * h - 2.2                   # knot m at t_m
    q = np.zeros((4, O, D), np.float64)           # poly coeffs from m=0..5
    for m in range(6):
        q[0] += -t[m] ** 3 * wp[:, :, m]
        q[1] += 3 * t[m] ** 2 * wp[:, :, m]
        q[2] += -3 * t[m] * wp[:, :, m]
        q[3] += wp[:, :, m]
    head = np.zeros((D, 20), np.float64)
    head[:, 0:2] = base_w.T                       # silu chunk
    for j in range(4):                            # ones, x, x^2, x^3
        head[:, 2 * (1 + j):2 * (1 + j) + 2] = q[j].T
    for k in range(5):                            # relu^3 knots m=6..10
        head[:, 2 * (5 + k):2 * (5 + k) + 2] = wp[:, :, 6 + k].T
    return head.astype(np.float32)


def _fold_weights(inp):
    wrs = inp["w_root_se"] + inp["w_root_ue"]
    wbigT = (wrs @ inp["w_email"]).T.copy()                     # [768, 128]
    mcagg = np.zeros((9, 128), np.float32)
    mcagg[0:8] = (inp["w_rel_ue"] @ inp["w_url"]).T             # url feats
    mcagg[8] = inp["w_rel_se"] @ inp["w_sender"][:, 0]          # sender feat
    # biases fold to a constant vector; this problem has all-zero biases.
    bias = (inp["b_rel_se"] + inp["b_rel_ue"] + wrs @ inp["b_email"])
    assert (np.all(inp["b_sender"] == 0) and np.all(inp["b_url"] == 0)
            and np.all(bias == 0)), "nonzero biases need the count path"
    head = _head_weights(inp["base_w"], inp["spline_w"])
    return wbigT, mcagg, head


def _wrap_idx16(flat):
    """int16 slot list -> [128, n/16] wrapped in 16 partitions, tiled to 128."""
    n = flat.shape[0]
    a = flat.astype(np.int16).reshape(n // 16, 16).T            # [16, n/16]
    return np.tile(a, (8, 1))


def _prep_edges(inp):
    """Flat per-class dst-sorted streams, x64-padded at 512-node pages.

    Per class returns: idx [8][128, E/16] i16; oh [8][128, nblk*512] f8;
    chunks [(idx_off_cols, nidx, pages)] where pages = per-page part lists
    [(local_grp, row_lo, row_hi, blk)] in chunk-local coordinates.
    """
    cls_edges = []
    cls_edges.append((inp["se_src"], inp["se_dst"], NS))                 # S
    ua = inp["ue_src"] < URL_SPLIT
    cls_edges.append((inp["ue_src"][ua], inp["ue_dst"][ua], URL_SPLIT))  # A
    cls_edges.append((inp["ue_src"][~ua] - URL_SPLIT, inp["ue_dst"][~ua],
                      NU - URL_SPLIT))                                   # B

    out = []
    for ci, (src, dst, zrow) in enumerate(cls_edges):
        percore = []
        npage = np.zeros((N_CORES, PAGES), np.int64)
        for c in range(N_CORES):
            sel = (dst >= c * NSH) & (dst < (c + 1) * NSH)
            s, d = src[sel], dst[sel] - c * NSH
            order = np.argsort(d, kind="stable")
            s, d = s[order], d[order]
            percore.append((s, d))
            npage[c] = np.bincount(d // 512, minlength=PAGES)
        e_p = 64 * np.ceil(npage.max(axis=0) / 64).astype(np.int64)
        etot = int(e_p.sum())
        assert etot % 64 == 0
        etot_r = 128 * ((etot + 127) // 128)      # idx stream rounding
        off_p = np.concatenate([[0], np.cumsum(e_p)])

        # chunk structure: consecutive pages, <= CH_MAX edges
        chunks = []           # (p0, p1, off0)
        p0 = 0
        while p0 < PAGES:
            p1 = p0 + 1
            n = int(e_p[p0])
            while p1 < PAGES and n + e_p[p1] <= CH_MAX:
                n += int(e_p[p1]); p1 += 1
            chunks.append((p0, p1, int(off_p[p0]), n))
            p0 = p1
        # per-chunk page part lists (chunk-local rows) + global block ids.
        # Each part gets a static 256-col node window; the first part of a
        # page is full-width (512) and starts the psum accumulation group.
        ncol = 0
        chunk_meta = []
        for (cp0, cp1, off0, n) in chunks:
            pages = []
            for p in range(cp0, cp1):
                nspan = min(512, NSH - 512 * p)
                parts = []
                pos = int(off_p[p]) - off0
                end = pos + int(e_p[p])
                ep = int(e_p[p])
                ps0 = pos
                while pos < end:
                    g, lo = pos // 128, pos % 128
                    hi = min(128, lo + (end - pos))
                    if not parts:
                        w0, wid = 0, 512
                    else:
                        mid = (pos + (pos + hi - lo)) / 2 - ps0
                        w0 = int(round(mid * nspan / ep - 128.0))
                        w0 = max(0, min(256, w0))
                        wid = 256
                    parts.append((g, lo, hi, ncol, w0, wid))
                    ncol += wid
                    pos += hi - lo
                pages.append(tuple(parts))
            chunk_meta.append((off0 // 16, n, tuple(pages)))

        # window-low per page-local slot (static across cores)
        wlo = {}
        for (ioff, n, pages) in chunk_meta:
            for parts in pages:
                base = parts[0][0] * 128 + parts[0][1]
                for (g, lo, hi, blkc, w0, wid) in parts:
                    for sl in range(g * 128 + lo, g * 128 + hi):
                        wlo[ioff * 16 + sl] = (w0, g * 128 + lo - base)
        idxs, ohs = [], []
        for c in range(N_CORES):
            s, d = percore[c]
            slots = np.full(etot_r, zrow, np.int32)
            pagecol = np.full(etot_r, -1, np.int64)
            pstart = np.concatenate([[0], np.cumsum(npage[c])])
            for p in range(PAGES):
                a, b = pstart[p], pstart[p + 1]
                o, ep = int(off_p[p]), int(e_p[p])
                # greedy banded placement: edge at slot sl needs
                # node - 512p in [w0(sl), w0(sl)+wid)
                sl = 0
                for i in range(a, b):
                    node = int(d[i]) - 512 * p
                    while True:
                        w0, po = wlo[o + sl]
                        wid = 512 if po == 0 else 256
                        if node < w0 + wid:
                            break
                        sl += 1
                        assert sl < ep, "band overflow"
                    assert node >= w0, "band underflow"
                    slots[o + sl] = s[i]
                    pagecol[o + sl] = node
                    sl += 1
                    assert sl <= ep
            idxs.append(_wrap_idx16(slots))
            oh = np.zeros((128, ncol), F8)
            for (ioff, n, pages) in chunk_meta:
                off0 = ioff * 16
                for parts in pages:
                    for (g, lo, hi, blkc, w0, wid) in parts:
                        for row in range(lo, hi):
                            e = off0 + g * 128 + row
                            col = pagecol[e]
                            if col >= 0:
                                assert w0 <= col < w0 + wid, (col, w0, wid)
                                oh[row, blkc + (col - w0)] = 1
            ohs.append(oh)
        out.append(dict(idx=idxs, oh=ohs, zrow=zrow, etot=etot_r, nblk=ncol,
                        chunks=tuple(chunk_meta)))
    return out


# ----------------------------------------------------------------- device build
def _build(meta):
    """meta: per class (etot, nblk, chunks); cross-core static."""
    nc = bacc.Bacc("TRN2", target_bir_lowering=False, debug=False,
                   num_devices=N_CORES)
    dt = lambda n, s, d, k: nc.dram_tensor(n, s, d, kind=k).ap()
    xT = dt("xT", [KIN, NP], BF16, "ExternalInput")
    tabs, idxd, ohd = [], [], []
    nrows = (NS + 1, URL_SPLIT + 1, NU - URL_SPLIT + 1)
    for ci in range(3):
        etot, nblk = meta[ci][0], meta[ci][1]
        tabs.append(dt(f"tab{ci}", [nrows[ci], ELEM], BF16, "ExternalInput"))
        idxd.append(dt(f"idx{ci}", [128, etot // 16], I16, "ExternalInput"))
        ohd.append(dt(f"oh{ci}", [128, nblk], FP8, "ExternalInput"))
    wbigT = dt("wbigT", [KIN, HID], BF16, "ExternalInput")
    mcagg = dt("mcagg", [9, HID], BF16, "ExternalInput")
    whead = dt("whead", [HID, 20], F32, "ExternalInput")
    whq0 = dt("whq0", [2, 1], F32, "ExternalInput")
    whsil = dt("whsil", [HID, 2], F16, "ExternalInput")
    outT = dt("outT", [2, NP], F32, "ExternalOutput")

    MAXG = CH_MAX // 128 + 2
    with tile.TileContext(nc) as tc:
        import contextlib
        with contextlib.ExitStack() as ctx:
            persist = ctx.enter_context(tc.tile_pool(name="persist", bufs=1))
            gpool = ctx.enter_context(tc.tile_pool(name="gath", bufs=2))
            opool = ctx.enter_context(tc.tile_pool(name="oh", bufs=1))
            xpool = ctx.enter_context(tc.tile_pool(name="x", bufs=2))
            ew = ctx.enter_context(tc.tile_pool(name="ew", bufs=2))
            psA = ctx.enter_context(tc.tile_pool(name="psA", bufs=2,
                                                 space="PSUM"))
            psB = ctx.enter_context(tc.tile_pool(name="psB", bufs=2,
                                                 space="PSUM"))
            psO = ctx.enter_context(tc.tile_pool(name="psO", bufs=2,
                                                 space="PSUM"))

            # ---- persistent small tensors
            wb = persist.tile([128, NKC * HID], BF16)
            nc.sync.dma_start(
                out=wb[:].rearrange("p (c h) -> p c h", c=NKC),
                in_=wbigT.rearrange("(c p) h -> p c h", p=128))
            mcA = persist.tile([9, HID], BF16)
            nc.sync.dma_start(out=mcA[:], in_=mcagg)
            wh = persist.tile([HID, 20], F32)
            nc.sync.dma_start(out=wh[:], in_=whead)
            wq0 = persist.tile([2, 1], F32)
            nc.sync.dma_start(out=wq0[:], in_=whq0)
            whs = persist.tile([HID, 2], F16)
            nc.sync.dma_start(out=whs[:], in_=whsil)
            kbias = persist.tile([128, 5], F32)
            for k, tk in enumerate(KNOTS):
                nc.vector.memset(kbias[:, k:k + 1], -tk)

            # ---- phase B (per 512-node page)
            def phase_b(p, pg):
                ns = slice(p * 512, (p + 1) * 512)
                xs = xpool.tile([128, NKC * 512], BF16, tag="xs")
                nc.sync.dma_start(
                    out=xs[:].rearrange("q (c n) -> q c n", c=NKC),
                    in_=xT[:, ns].rearrange("(c q) n -> q c n", q=128))
                pP = psB.tile([128, 512], F32, space="PSUM", tag="pP")
                for k in range(NKC):
                    nc.tensor.matmul(
                        out=pP[:], lhsT=wb[:, k * HID:(k + 1) * HID],
                        rhs=xs[:, k * 512:(k + 1) * 512],
                        start=(k == 0), stop=False)
                nc.tensor.matmul(out=pP[:], lhsT=mcA[:], rhs=pg[0:9, :],
                                 start=False, stop=True)

                # KAN head; f32 chunks. DVE ops 1-port only.
                xt = ew.tile([128, 512], F32, tag="xt")     # clamp(h,0,2.2)
                nc.vector.tensor_scalar(out=xt[:], in0=pP[:], scalar1=0.0,
                                        scalar2=XCLAMP,
                                        op0=mybir.AluOpType.max,
                                        op1=mybir.AluOpType.min)
                sil = ew.tile([128, 512], F32, tag="sil")
                nc.scalar.activation(sil[:], pP[:],
                                     mybir.ActivationFunctionType.Silu)
                rsil = ew.tile([128, 512], F16, tag="rsil")
                nc.scalar.activation(rsil[:], sil[:],
                                     mybir.ActivationFunctionType.Relu)
                x2 = ew.tile([128, 512], F32, tag="x2")
                nc.scalar.activation(x2[:], xt[:],
                                     mybir.ActivationFunctionType.Square)
                x3 = ew.tile([128, 512], F32, tag="x3")
                nc.vector.tensor_tensor(out=x3[:], in0=x2[:], in1=xt[:],
                                        op=mybir.AluOpType.mult)
                pO = psO.tile([2, 512], F32, space="PSUM", tag="pO")
                nc.tensor.matmul(out=pO[:], lhsT=whs[:], rhs=rsil[:],
                                 start=True, stop=False)
                for j, ck in ((2, xt), (3, x2), (4, x3)):
                    nc.tensor.matmul(out=pO[:], lhsT=wh[:, 2 * j:2 * j + 2],
                                     rhs=ck[:], start=False, stop=False)
                for k in range(5):
                    r = ew.tile([128, 512], F32, tag="r")
                    nc.scalar.activation(r[:], xt[:],
                                         mybir.ActivationFunctionType.Relu,
                                         bias=kbias[:, k:k + 1])
                    r2 = ew.tile([128, 512], F32, tag="r2")
                    nc.vector.tensor_tensor(out=r2[:], in0=r[:], in1=r[:],
                                            op=mybir.AluOpType.mult)
                    r3 = ew.tile([128, 512], F32, tag="r3")
                    nc.vector.tensor_tensor(out=r3[:], in0=r2[:], in1=r[:],
                                            op=mybir.AluOpType.mult)
                    nc.tensor.matmul(out=pO[:],
                                     lhsT=wh[:, 10 + 2 * k:12 + 2 * k],
                                     rhs=r3[:], start=False, stop=(k == 4))
                ot = ew.tile([2, 512], F32, tag="ot")
                nc.vector.tensor_scalar_add(out=ot[:], in0=pO[:],
                                            scalar1=wq0[:])
                nc.sync.dma_start(out=outT[:, ns], in_=ot[:])

            # ---- main loop: gathers stream in page-aligned chunks;
            #      per page: scatter part-matmuls -> psum -> phase B.
            cptr = [0, 0, 0]
            cur = [None, None, None]        # (gt, oh, pages, next_page_idx)
            nextp = [0, 0, 0]               # first page of next chunk
            for p in range(PAGES):
                for ci in range(3):
                    etot, nblk, chunks = meta[ci]
                    if nextp[ci] == p:
                        ioff, n, pages = chunks[cptr[ci]]
                        ng = (n + 127) // 128
                        ncols = (n + 15) // 16
                        isb = gpool.tile([128, CH_MAX // 16 + 8], I16,
                                         tag=f"i{ci}")
                        nc.sync.dma_start(
                            out=isb[:, :ncols],
                            in_=idxd[ci][:, ioff:ioff + ncols])
                        gt = gpool.tile([128, MAXG, ELEM], BF16, tag=f"g{ci}")
                        if n % 128:
                            nc.vector.memset(gt[:, ng - 1, :], 0.0)
                        nc.gpsimd.dma_gather(
                            out_ap=gt[:, :ng, :], in_ap=tabs[ci],
                            idxs_ap=isb[:, :ncols],
                            num_idxs=n, num_idxs_reg=n, elem_size=ELEM,
                            single_packet=False)
                        blk0 = pages[0][0][3]
                        ncols_oh = sum(pp[-1][3] + pp[-1][5]
                                       for pp in pages[-1:]) - blk0
                        oh = opool.tile([128, (MAXG + 8) * 512], FP8,
                                        tag=f"o{ci}")
                        nc.sync.dma_start(
                            out=oh[:, :ncols_oh],
                            in_=ohd[ci][:, blk0:blk0 + ncols_oh])
                        cur[ci] = (gt, oh, pages, blk0)
                        cptr[ci] += 1
                        nextp[ci] = p + len(pages)
                pg = psA.tile([16, 512], F32, space="PSUM", tag="pg")
                mms = []
                for ci in range(3):
                    gt, oh, pages, blk0 = cur[ci]
                    pidx = p - (nextp[ci] - len(pages))
                    for (g, lo, hi, blkc, w0, wid) in pages[pidx]:
                        mms.append((gt, oh, g, blkc - blk0, w0, wid))
                for i, (gt, oh, g, blkc, w0, wid) in enumerate(mms):
                    nc.tensor.matmul(
                        out=pg[0:9, w0:w0 + wid], lhsT=gt[:, g, 0:9],
                        rhs=oh[:, blkc:blkc + wid],
                        start=(i == 0), stop=(i == len(mms) - 1))
                pgs = ew.tile([9, 512], BF16, tag="pgs")
                nc.scalar.copy(out=pgs[:], in_=pg[0:9, :])
                phase_b(p, pgs)

    nc.compile()
    return nc


# ----------------------------------------------------------------- entry point
def kernel(**inp):
    inp = {k: np.asarray(v) for k, v in inp.items()}
    wbigT, mcagg, head = _fold_weights(inp)
    eprep = _prep_edges(inp)

    meta = tuple((e["etot"], e["nblk"], e["chunks"]) for e in eprep)
    if meta not in _CACHE:
        _CACHE[meta] = _build(meta)
    nc = _CACHE[meta]

    tabS = np.zeros((NS + 1, ELEM), BF)
    tabS[:NS, 8] = inp["x_sender"][:, 0].astype(BF)
    tabA = np.zeros((URL_SPLIT + 1, ELEM), BF)
    tabA[:URL_SPLIT, 0:8] = inp["x_url"][:URL_SPLIT].astype(BF)
    tabB = np.zeros((NU - URL_SPLIT + 1, ELEM), BF)
    tabB[:NU - URL_SPLIT, 0:8] = inp["x_url"][URL_SPLIT:].astype(BF)
    q0 = head[:, 2:4].sum(axis=0).reshape(2, 1)

    in_maps = []
    for c in range(N_CORES):
        xsh = np.zeros((KIN, NP), BF)
        xsh[:, :NSH] = inp["x_email"][c * NSH:(c + 1) * NSH].T.astype(BF)
        m = {"xT": xsh, "tab0": tabS, "tab1": tabA, "tab2": tabB,
             "wbigT": wbigT.astype(BF), "mcagg": mcagg.astype(BF),
             "whead": head, "whq0": q0,
             "whsil": head[:, 0:2].astype(np.float16)}
        for ci in range(3):
            m[f"idx{ci}"] = eprep[ci]["idx"][c]
            m[f"oh{ci}"] = eprep[ci]["oh"][c]
        in_maps.append(m)

    global _LAST_RESULT
    trace = os.environ.get("KERNEL_TRACE", "0") == "1"
    res = run_bass_kernel_spmd(nc, in_maps, core_ids=list(range(N_CORES)),
                               trace=trace)
    _LAST_RESULT = res
    out = np.empty((NE, 2), np.float32)
    for c in range(N_CORES):
        out[c * NSH:(c + 1) * NSH] = res.results[c]["outT"][:, :NSH].T
    return out
